# revision 26
# baseline (speedup 1.0000x reference)
"""ErnieLayout self-attention on 8 Trainium2 NeuronCores (Bass/Tile).

Problem shapes (hardcoded): B=4, S=1024, H=768, NH=12, HD=64.
Sharding: core c -> (batch b = c//2, head-half hh = c%2, i.e. 6 heads).
Each core computes attention for its 6 heads of one batch element and
writes the [S, 384] column slice of that batch's output.

v5 design (per-core, mixed precision, scores kept transposed):
  rel12 = rel_pos + rel_2d_pos: fp32 strips stream on the sync (HWDGE)
  queue -- the only DMA path that sustains ~400 GB/s here (SWDGE
  cast/accum DMAs cap at ~150 GB/s write-side and XBAR-transpose DMAs
  serialize against every other in-flight DMA; both were measured on
  HW and rejected).  A DVE pass adds each fp32 pair into fp16 strip
  tiles; r1/r2 staging 8 deep so the strip DMAs run a full head ahead
  of the adds, 28 fp16 strip tiles = 3.5 heads of prefetch.

  setup:  mask/bias vectors ride the gpsimd (SWDGE) queue first, then
          the W tiles -- the sync queue carries only X + rel + out, so
          the rel stream starts ~10us earlier.  X casts to fp16
          alternate ACT/DVE, W casts ride ACT; all transposed on the
          PE (fp16).  Q^T = (Wq_s @ X^T + bq)/8, K^T = Wk_s @ X^T +
          bk (fp16 matmuls, fp32 PSUM accumulate); V = X @ Wv_s^T
          (+ bv), fp16 with a ones column appended (col 64 -> softmax
          denominator for free).  mask/bias are loaded as [rows, 128]
          tiles (contiguous 512B descriptors) and PE-transposed after
          the X^T section -- no 4B-element gather DMAs anywhere and
          nothing early in the PE stream waits on the W queue.
  scores: per (head, ktile, q-chunk):
          psum[k=128, q=512] = K^T.T @ Q^T  (fp16, 1 cyc/row)
          psum += rel12[q,ktile]^T via matmul(lhsT=rel12_f16, rhs=I)
          pT = exp(psum + maskbias) -> fp16 (ACT per-partition bias;
          masked keys get FLT_MIN so exp underflows to exactly 0).
          spsum ring of 6 banks lets the PE run ~1.5 ktiles ahead of
          the exps instead of lock-stepping with ACT.
  PV:     ctx^T[d|1, q-chunk] += V_aug[kt].T @ pT[kt], emitted one kt
          behind the exp that produces pT[kt] (the PE never waits on
          ACT, and the post-last-DMA tail is ~one kt of work).
  fin:    ctx^T -> SBUF fp16 on the DVE (keeps ACT free for exps),
          back-transposed on the PE in fp16 (1 cyc/row; ctx values are
          O(1e3), well inside fp16), evacuated from PSUM to SBUF in one
          DVE copy so the recip (DVE) / scale (ACT) chain never
          ping-pongs on a PSUM bank and the score-psum ring frees
          immediately (this was a measured ~3.7us/head PE stall);
          out[q, h*64+d] = ctx[q, d] * (1 / ctx[q, 64]); the finalize
          of head h is emitted inside head h+1's loop, and the last
          head's output DMAs are interleaved with its scales.
Precision: fp16 carries 10 mantissa bits -> final rel err ~1e-3.
"""

import os
import sys

import numpy as np

for _p in ("/opt/trn_rl_repo",):
    if _p not in sys.path and os.path.isdir(_p):
        sys.path.append(_p)

import concourse.bass as bass
import concourse.mybir as mybir
import concourse.tile as tile
from concourse import bacc
from concourse.bass_utils import run_bass_kernel_spmd
from concourse.masks import make_identity

F32 = mybir.dt.float32
F16 = mybir.dt.float16
I32 = mybir.dt.int32
AF = mybir.ActivationFunctionType
NEG = float(np.finfo(np.float32).min)

P = 128
S = 1024
NH = 6        # heads per core
HD = 64
HIN = 768     # model dim (contraction for projections)
HOUT = NH * HD  # 384, per-core projection width
KT = S // P   # 8 key tiles
QT = S // P   # 8 query tiles
VW = HD + 1   # 65: V columns + ones column
VS = VW + 1   # 66: psum stride per transposed block (4B-aligned fp16)


def _build_kernel_body(tc, aps):
    import contextlib

    nc = tc.nc
    x_ap = aps["x"]
    mask_ap = aps["mask"]
    rel1_ap = aps["rel1"]
    rel2_ap = aps["rel2"]
    out_ap = aps["out"]

    with contextlib.ExitStack() as ctx:
        const = ctx.enter_context(tc.tile_pool(name="const", bufs=1))

        ident = const.tile([P, P], F16)
        make_identity(nc, ident)
        ident32 = const.tile([P, P], F32)
        nc.scalar.copy(ident32[:], ident[:])

        # long-lived tensors
        qt_pool = ctx.enter_context(tc.tile_pool(name="qT", bufs=3))
        kt_pool = ctx.enter_context(tc.tile_pool(name="kT", bufs=3))
        v_pool = ctx.enter_context(tc.tile_pool(name="v", bufs=8))

        qT = [qt_pool.tile([P, S], F16, tag="qT", name=f"qT{i}") for i in range(3)]
        kT = [kt_pool.tile([P, S], F16, tag="kT", name=f"kT{i}") for i in range(3)]
        v_tiles = [
            v_pool.tile([P, NH, VW], F16, tag="v", name=f"v{i}") for i in range(8)
        ]

        # rel stream pools first: their SBUF is disjoint from phase-1 pools,
        # so rel DMA + DVE adds run from t=0 and deep fp16 buffering keeps
        # the DMA queues fed across head boundaries.
        r1_pool = ctx.enter_context(tc.tile_pool(name="r1", bufs=8))
        r2_pool = ctx.enter_context(tc.tile_pool(name="r2", bufs=8))
        rbf_pool = ctx.enter_context(tc.tile_pool(name="rbf", bufs=28))

        # ---------------- phase 1: load, cast, transpose, project ----------
        with contextlib.ExitStack() as ph1:
            xload = ph1.enter_context(tc.tile_pool(name="xload", bufs=2))
            wload = ph1.enter_context(tc.tile_pool(name="wload", bufs=2))
            x16_pool = ph1.enter_context(tc.tile_pool(name="x16", bufs=8))
            w16_pool = ph1.enter_context(tc.tile_pool(name="w16", bufs=4))
            xt_pool = ph1.enter_context(tc.tile_pool(name="xT", bufs=6))
            wt_pool = ph1.enter_context(tc.tile_pool(name="wT", bufs=18))
            psum1 = ph1.enter_context(tc.tile_pool(name="psum1", bufs=3, space="PSUM"))
            psum1b = ph1.enter_context(
                tc.tile_pool(name="psum1b", bufs=2, space="PSUM")
            )

            # mask and projection-bias vectors ride the gpsimd queue
            # FIRST (tiny; ahead of the 3.5MB of W); loaded as [rows, 128]
            # tiles (contiguous 512B per partition), PE-transposed later.
            mrow_i = const.tile([KT, P], I32, tag="mrow_i")
            nc.gpsimd.dma_start(mrow_i[:], mask_ap.rearrange("(a p) -> a p", p=P))
            mrow = const.tile([KT, P], F32, tag="mrow")
            nc.vector.tensor_copy(mrow[:], mrow_i[:])
            brow = {}
            for wname in ("q", "k"):
                br = const.tile([3, P], F32, tag=f"brow{wname}")
                nc.gpsimd.dma_start(
                    br[:], aps[f"b{wname}"].rearrange("(a p) -> a p", p=P)
                )
                brow[wname] = br
            bv_bc = const.tile([P, NH, HD], F32)
            nc.gpsimd.dma_start(
                bv_bc[:],
                aps["bv"].rearrange("(h d) -> h d", d=HD)[None].to_broadcast(
                    (P, NH, HD)
                ),
            )

            # X tiles [128, 768] -> fp16, casts alternating ACT/DVE
            x16 = []
            for t in range(8):
                xt_ = xload.tile([P, HIN], F32, tag="x")
                nc.sync.dma_start(xt_[:], x_ap[t * P:(t + 1) * P, :])
                x16_t = x16_pool.tile([P, HIN], F16, tag="x16", name=f"x16_{t}")
                if t % 2 == 0:
                    nc.scalar.copy(x16_t[:], xt_[:])
                else:
                    nc.vector.tensor_copy(x16_t[:], xt_[:])
                x16.append(x16_t)

            # W loads (gpsimd queue, parallel with X/rel on sync) + ACT casts
            w16 = {}
            for wname in ("q", "k", "v"):
                w_ap = aps[f"w{wname}"]
                w16s = []
                for d in range(3):
                    wt_ = wload.tile([P, HIN], F32, tag="wload")
                    nc.gpsimd.dma_start(wt_[:], w_ap[d * P:(d + 1) * P, :])
                    w16_t = w16_pool.tile(
                        [P, HIN], F16, tag="w16", name=f"w16{wname}_{d}"
                    )
                    nc.scalar.copy(w16_t[:], wt_[:])
                    w16s.append(w16_t)
                w16[wname] = w16s

            # X^T: 6 fp16 tiles [128, 1024] (h-chunk on partitions);
            # psum -> SBUF copies on the DVE (16-bit, 2x rate)
            xT = []
            for hc in range(6):
                pt = psum1.tile([P, S], F16, tag="xtp")  # 1 bank (fp16)
                for t in range(8):
                    nc.tensor.transpose(
                        pt[:, t * P:(t + 1) * P],
                        x16[t][:, hc * P:(hc + 1) * P],
                        ident[:],
                    )
                xt_t = xt_pool.tile([P, S], F16, tag="xT")
                nc.vector.tensor_copy(xt_t[:], pt[:])
                xT.append(xt_t)

            pm = psum1b.tile([P, 512], F32, tag="projp", name="pm")
            nc.tensor.transpose(pm[:, 0:KT], mrow[:], ident32[:KT, :KT])
            nc.tensor.transpose(pm[:, 16:19], brow["q"][:], ident32[:3, :3])
            nc.tensor.transpose(pm[:, 24:27], brow["k"][:], ident32[:3, :3])
            maskb = const.tile([P, KT], F32)
            nc.vector.tensor_scalar_mul(maskb[:], pm[:, 0:KT], NEG)
            bias_sb = {}
            bq_t = const.tile([P, 3], F32, tag="bq")
            nc.vector.tensor_scalar_mul(bq_t[:], pm[:, 16:19], 0.125)
            bias_sb["q"] = bq_t
            bk_t = const.tile([P, 3], F32, tag="bk")
            nc.vector.tensor_copy(bk_t[:], pm[:, 24:27])
            bias_sb["k"] = bk_t

            # W^T slices (fp16): wT[(w, hc)] = [128, 384] (copies on ACT)
            wT = {}
            for wname in ("q", "k", "v"):
                w16s = w16[wname]
                for hc in range(6):
                    pw = psum1b.tile([P, 512], F16, tag="ps1b", name="pw")[:, :HOUT]
                    for d in range(3):
                        nc.tensor.transpose(
                            pw[:, d * P:(d + 1) * P],
                            w16s[d][:, hc * P:(hc + 1) * P],
                            ident[:],
                        )
                    wt_t = wt_pool.tile([P, HOUT], F16, tag="wT")
                    nc.scalar.copy(wt_t[:], pw[:])
                    wT[(wname, hc)] = wt_t

            # Q^T, K^T projections: fp16 matmuls, fp32 PSUM
            for wname, dest, scale in (("q", qT, 0.125), ("k", kT, 1.0)):
                for d in range(3):
                    for tch in range(2):
                        pp = psum1b.tile([P, 512], F32, tag="projp")
                        for hc in range(6):
                            nc.tensor.matmul(
                                pp[:],
                                wT[(wname, hc)][:, d * P:(d + 1) * P],
                                xT[hc][:, tch * 512:(tch + 1) * 512],
                                start=(hc == 0),
                                stop=(hc == 5),
                            )
                        nc.scalar.activation(
                            dest[d][:, tch * 512:(tch + 1) * 512],
                            pp[:],
                            AF.Identity,
                            bias=bias_sb[wname][:, d:d + 1],
                            scale=scale,
                        )

            # V projection: out [t-tile 128, 384] fp16 + ones column
            for t in range(8):
                pv = psum1b.tile([P, 512], F32, tag="projp", name="pv")[:, :HOUT]
                for hc in range(6):
                    nc.tensor.matmul(
                        pv[:],
                        xT[hc][:, t * P:(t + 1) * P],
                        wT[("v", hc)][:],
                        start=(hc == 0),
                        stop=(hc == 5),
                    )
                nc.vector.memset(v_tiles[t][:], 1.0)
                # copy + bias add (bv broadcast along partitions)
                nc.vector.tensor_add(
                    v_tiles[t][:, :, 0:HD],
                    pv[:].rearrange("p (h d) -> p h d", d=HD),
                    bv_bc[:],
                )

        # ---------------- phase 2: attention per head ----------------
        out_pool = ctx.enter_context(tc.tile_pool(name="outst", bufs=8))
        out_stage = [
            out_pool.tile([P, HOUT], F32, tag="outst", name=f"outst{i}")
            for i in range(8)
        ]
        pt_pool = ctx.enter_context(tc.tile_pool(name="pT", bufs=8))
        fin_pool = ctx.enter_context(tc.tile_pool(name="fin", bufs=8))
        ctt_pool = ctx.enter_context(tc.tile_pool(name="ctt", bufs=4))
        ctsb_pool = ctx.enter_context(tc.tile_pool(name="ctsb", bufs=2))
        spsum = ctx.enter_context(tc.tile_pool(name="spsum", bufs=6, space="PSUM"))
        vpsum = ctx.enter_context(tc.tile_pool(name="vpsum", bufs=2, space="PSUM"))

        def load_strips(h):
            """rel1+rel2 -> fp16 strip tiles for head h (DVE add)."""
            strips = []
            for qq in range(8):
                r1 = r1_pool.tile([P, S], F32, tag="r1")
                nc.sync.dma_start(
                    r1[:],
                    rel1_ap[h].rearrange("(qt p) k -> p qt k", p=P)[:, qq, :],
                )
                r2 = r2_pool.tile([P, S], F32, tag="r2")
                nc.sync.dma_start(
                    r2[:],
                    rel2_ap[h].rearrange("(qt p) k -> p qt k", p=P)[:, qq, :],
                )
                rb = rbf_pool.tile([P, S], F16, tag="rbf", name=f"rbf{h}_{qq}")
                nc.vector.tensor_add(rb[:], r1[:], r2[:])
                strips.append(rb)
            return strips

        def emit_finalize(h, ctxT_ps, last=False):
            """Epilogue for head h: ctx^T out of PSUM as fp16 (DVE copy),
            back-transpose on the PE (fp16, 1 cyc/row) into a dedicated
            1-bank psum, evacuate once to SBUF (DVE), then divide by the
            denominator column reading SBUF only -- the score-psum ring
            and the PE stream never wait on the recip/scale chain.
            Deferred one head so the in-order PE stream never stalls."""
            ctxT_sb = []
            for qch in range(2):
                t16 = ctt_pool.tile([VW, 512], F16, tag="ctxT16",
                                    name=f"ctxTs{h}_{qch}")
                nc.scalar.copy(t16[:], ctxT_ps[qch][:])
                ctxT_sb.append(t16)
            ctx_ps = spsum.tile([P, QT * VS], F16, tag="sT", name=f"ctxp{h}")
            for qt in range(QT):
                sl = qt * VS
                nc.tensor.transpose(
                    ctx_ps[:, sl:sl + VW],
                    ctxT_sb[qt // 4][:, (qt % 4) * P:(qt % 4 + 1) * P],
                    ident[:VW, :VW],
                )
            ctx_sb = ctsb_pool.tile([P, QT * VS], F16, tag="ctsb",
                                    name=f"ctxsb{h}")
            nc.vector.tensor_copy(ctx_sb[:], ctx_ps[:])
            for qt in range(QT):
                sl = qt * VS
                rc = fin_pool.tile([P, 1], F32, tag="recip")
                nc.vector.reciprocal(rc[:], ctx_sb[:, sl + HD:sl + HD + 1])
                nc.scalar.activation(
                    out_stage[qt][:, h * HD:(h + 1) * HD],
                    ctx_sb[:, sl:sl + HD],
                    AF.Identity,
                    scale=rc[:],
                )
                if last:
                    nc.sync.dma_start(
                        out_ap[qt * P:(qt + 1) * P, :], out_stage[qt][:]
                    )

        pending_fin = None
        for h in range(NH):
            strips = load_strips(h)
            dt, rem = divmod(h, 2)
            d0 = rem * HD
            qTh = qT[dt][d0:d0 + HD, :]
            kTh = kT[dt][d0:d0 + HD, :]

            ctxT_ps = [
                vpsum.tile([VW, 512], F32, tag="ctxT", name=f"ctxT{h}_{i}")
                for i in range(2)
            ]

            pT_prev = None
            for kt in range(KT):
                kl = slice(kt * P, (kt + 1) * P)
                pT_cur = pt_pool.tile([P, S], F16, tag="pT", name=f"pT{h}_{kt}")
                for qch in range(2):
                    ps = spsum.tile([P, 512], F32, tag="sT")
                    # qk^T
                    nc.tensor.matmul(
                        ps[:],
                        kTh[:, kl],
                        qTh[:, qch * 512:(qch + 1) * 512],
                        start=True,
                        stop=False,
                    )
                    # += rel12^T (transposing adds via fp16 identity rhs)
                    for j in range(4):
                        qt = qch * 4 + j
                        nc.tensor.matmul(
                            ps[:, j * P:(j + 1) * P],
                            strips[qt][:, kl],
                            ident[:],
                            start=False,
                            stop=(j == 3),
                        )
                    # exp(scores + mask bias) -> fp16 probs
                    nc.scalar.activation(
                        pT_cur[:, qch * 512:(qch + 1) * 512],
                        ps[:],
                        AF.Exp,
                        bias=maskb[:, kt:kt + 1],
                        scale=1.0,
                    )
                # PV for the previous kt (one behind: the PE never waits
                # on the exps of the current kt)
                if pT_prev is not None:
                    for qch in range(2):
                        nc.tensor.matmul(
                            ctxT_ps[qch][:],
                            v_tiles[kt - 1][:, h, :],
                            pT_prev[:, qch * 512:(qch + 1) * 512],
                            start=(kt == 1),
                            stop=False,
                        )
                pT_prev = pT_cur
                if kt == 0 and pending_fin is not None:
                    emit_finalize(*pending_fin)
                    pending_fin = None

            # last kt's PV closes the accumulation
            for qch in range(2):
                nc.tensor.matmul(
                    ctxT_ps[qch][:],
                    v_tiles[KT - 1][:, h, :],
                    pT_prev[:, qch * 512:(qch + 1) * 512],
                    start=False,
                    stop=True,
                )

            pending_fin = (h, ctxT_ps)

        emit_finalize(*pending_fin, last=True)


def build_program():
    """Build and compile the per-core Bass program. Returns nc."""
    nc = bacc.Bacc(
        "TRN2",
        target_bir_lowering=False,
        debug=False,
        num_devices=8,
    )
    aps = {
        "x": nc.dram_tensor("x", [S, HIN], F32, kind="ExternalInput").ap(),
        "mask": nc.dram_tensor("mask", [S], I32, kind="ExternalInput").ap(),
        "rel1": nc.dram_tensor("rel1", [NH, S, S], F32, kind="ExternalInput").ap(),
        "rel2": nc.dram_tensor("rel2", [NH, S, S], F32, kind="ExternalInput").ap(),
        "wq": nc.dram_tensor("wq", [HOUT, HIN], F32, kind="ExternalInput").ap(),
        "wk": nc.dram_tensor("wk", [HOUT, HIN], F32, kind="ExternalInput").ap(),
        "wv": nc.dram_tensor("wv", [HOUT, HIN], F32, kind="ExternalInput").ap(),
        "bq": nc.dram_tensor("bq", [HOUT], F32, kind="ExternalInput").ap(),
        "bk": nc.dram_tensor("bk", [HOUT], F32, kind="ExternalInput").ap(),
        "bv": nc.dram_tensor("bv", [HOUT], F32, kind="ExternalInput").ap(),
        "out": nc.dram_tensor("out", [S, HOUT], F32, kind="ExternalOutput").ap(),
    }
    with tile.TileContext(nc) as tc:
        _build_kernel_body(tc, aps)
    nc.compile()
    return nc


def make_in_maps(inputs):
    """Slice full inputs into the 8 per-core input maps."""
    hs = np.ascontiguousarray(np.asarray(inputs["hidden_states"], np.float32))
    am = np.asarray(inputs["attention_mask"]).astype(np.int32)
    rel1 = np.asarray(inputs["rel_pos"], np.float32)
    rel2 = np.asarray(inputs["rel_2d_pos"], np.float32)
    ws = {k: np.asarray(inputs["W" + k[-1]], np.float32) for k in ("wq", "wk", "wv")}
    bs = {k: np.asarray(inputs["b" + k[-1]], np.float32) for k in ("bq", "bk", "bv")}

    in_maps = []
    for c in range(8):
        b, hh = divmod(c, 2)
        hsl = slice(hh * NH, (hh + 1) * NH)
        csl = slice(hh * HOUT, (hh + 1) * HOUT)
        m = {
            "x": np.ascontiguousarray(hs[b]),
            "mask": np.ascontiguousarray(am[b, 0, 0]),
            "rel1": np.ascontiguousarray(rel1[b, hsl]),
            "rel2": np.ascontiguousarray(rel2[b, hsl]),
        }
        for k in ("wq", "wk", "wv"):
            m[k] = np.ascontiguousarray(ws[k][csl])
        for k in ("bq", "bk", "bv"):
            m[k] = np.ascontiguousarray(bs[k][csl])
        in_maps.append(m)
    return in_maps


def gather_output(results):
    out = np.empty((4, S, HIN), np.float32)
    for c in range(8):
        b, hh = divmod(c, 2)
        out[b, :, hh * HOUT:(hh + 1) * HOUT] = results[c]["out"]
    return out


_NC_CACHE = []


def kernel(**inputs):
    if not _NC_CACHE:
        _NC_CACHE.append(build_program())
    nc = _NC_CACHE[0]
    in_maps = make_in_maps(inputs)
    res = run_bass_kernel_spmd(nc, in_maps, list(range(8)))
    return gather_output(res.results)


# revision 27
# speedup vs baseline: 1.0281x; 1.0281x over previous
"""ErnieLayout self-attention on 8 Trainium2 NeuronCores (Bass/Tile).

Problem shapes (hardcoded): B=4, S=1024, H=768, NH=12, HD=64.
Sharding: core c -> (batch b = c//2, head-half hh = c%2, i.e. 6 heads).
Each core computes attention for its 6 heads of one batch element and
writes the [S, 384] column slice of that batch's output.

v5 design (per-core, mixed precision, scores kept transposed):
  rel12 = rel_pos + rel_2d_pos: fp32 strips stream on the sync (HWDGE)
  queue -- the only DMA path that sustains ~400 GB/s here (SWDGE
  cast/accum DMAs cap at ~150 GB/s write-side and XBAR-transpose DMAs
  serialize against every other in-flight DMA; both were measured on
  HW and rejected).  A DVE pass adds each fp32 pair into fp16 strip
  tiles; r1/r2 staging 8 deep so the strip DMAs run a full head ahead
  of the adds, 28 fp16 strip tiles = 3.5 heads of prefetch.

  setup:  mask/bias vectors ride the gpsimd (SWDGE) queue first, then
          the W tiles -- the sync queue carries only X + rel + out, so
          the rel stream starts ~10us earlier.  X casts to fp16
          alternate ACT/DVE, W casts ride ACT; all transposed on the
          PE (fp16).  Q^T = (Wq_s @ X^T + bq)/8, K^T = Wk_s @ X^T +
          bk (fp16 matmuls, fp32 PSUM accumulate); V = X @ Wv_s^T
          (+ bv), fp16 with a ones column appended (col 64 -> softmax
          denominator for free).  mask/bias are loaded as [rows, 128]
          tiles (contiguous 512B descriptors) and PE-transposed after
          the X^T section -- no 4B-element gather DMAs anywhere and
          nothing early in the PE stream waits on the W queue.
  scores: per (head, ktile, q-chunk):
          psum[k=128, q=512] = K^T.T @ Q^T  (fp16, 1 cyc/row)
          psum += rel12[q,ktile]^T via matmul(lhsT=rel12_f16, rhs=I)
          pT = exp(psum + maskbias) -> fp16 (ACT per-partition bias;
          masked keys get FLT_MIN so exp underflows to exactly 0).
          spsum ring of 6 banks lets the PE run ~1.5 ktiles ahead of
          the exps instead of lock-stepping with ACT.
  PV:     ctx^T[d|1, q-chunk] += V_aug[kt].T @ pT[kt], emitted one kt
          behind the exp that produces pT[kt] (the PE never waits on
          ACT, and the post-last-DMA tail is ~one kt of work).
  fin:    ctx^T -> SBUF fp16 on the DVE (keeps ACT free for exps),
          back-transposed on the PE in fp16 (1 cyc/row; ctx values are
          O(1e3), well inside fp16), evacuated from PSUM to SBUF in one
          DVE copy so the recip (DVE) / scale (ACT) chain never
          ping-pongs on a PSUM bank and the score-psum ring frees
          immediately (this was a measured ~3.7us/head PE stall);
          out[q, h*64+d] = ctx[q, d] * (1 / ctx[q, 64]); the finalize
          of head h is emitted inside head h+1's loop, and the last
          head's output DMAs are interleaved with its scales.
Precision: fp16 carries 10 mantissa bits -> final rel err ~1e-3.
"""

import os
import sys

import numpy as np

for _p in ("/opt/trn_rl_repo",):
    if _p not in sys.path and os.path.isdir(_p):
        sys.path.append(_p)

import concourse.bass as bass
import concourse.mybir as mybir
import concourse.tile as tile
from concourse import bacc
from concourse.bass_utils import run_bass_kernel_spmd
from concourse.masks import make_identity

F32 = mybir.dt.float32
F16 = mybir.dt.float16
I32 = mybir.dt.int32
AF = mybir.ActivationFunctionType
NEG = float(np.finfo(np.float32).min)

P = 128
S = 1024
NH = 6        # heads per core
HD = 64
HIN = 768     # model dim (contraction for projections)
HOUT = NH * HD  # 384, per-core projection width
KT = S // P   # 8 key tiles
QT = S // P   # 8 query tiles
VW = HD + 1   # 65: V columns + ones column
VS = VW + 1   # 66: psum stride per transposed block (4B-aligned fp16)


def _build_kernel_body(tc, aps):
    import contextlib

    nc = tc.nc
    x_ap = aps["x"]
    mask_ap = aps["mask"]
    rel1_ap = aps["rel1"]
    rel2_ap = aps["rel2"]
    out_ap = aps["out"]

    with contextlib.ExitStack() as ctx:
        const = ctx.enter_context(tc.tile_pool(name="const", bufs=1))

        ident = const.tile([P, P], F16)
        make_identity(nc, ident)
        ident32 = const.tile([P, P], F32)
        nc.scalar.copy(ident32[:], ident[:])

        # long-lived tensors
        qt_pool = ctx.enter_context(tc.tile_pool(name="qT", bufs=3))
        kt_pool = ctx.enter_context(tc.tile_pool(name="kT", bufs=3))
        v_pool = ctx.enter_context(tc.tile_pool(name="v", bufs=8))

        qT = [qt_pool.tile([P, S], F16, tag="qT", name=f"qT{i}") for i in range(3)]
        kT = [kt_pool.tile([P, S], F16, tag="kT", name=f"kT{i}") for i in range(3)]
        v_tiles = [
            v_pool.tile([P, NH, VW], F16, tag="v", name=f"v{i}") for i in range(8)
        ]

        # rel stream pools first: their SBUF is disjoint from phase-1 pools,
        # so rel DMA + DVE adds run from t=0 and deep fp16 buffering keeps
        # the DMA queues fed across head boundaries.
        r1_pool = ctx.enter_context(tc.tile_pool(name="r1", bufs=8))
        r2_pool = ctx.enter_context(tc.tile_pool(name="r2", bufs=8))
        rbf_pool = ctx.enter_context(tc.tile_pool(name="rbf", bufs=28))

        # ---------------- phase 1: load, cast, transpose, project ----------
        with contextlib.ExitStack() as ph1:
            xload = ph1.enter_context(tc.tile_pool(name="xload", bufs=2))
            wload = ph1.enter_context(tc.tile_pool(name="wload", bufs=2))
            x16_pool = ph1.enter_context(tc.tile_pool(name="x16", bufs=8))
            w16_pool = ph1.enter_context(tc.tile_pool(name="w16", bufs=4))
            xt_pool = ph1.enter_context(tc.tile_pool(name="xT", bufs=6))
            wt_pool = ph1.enter_context(tc.tile_pool(name="wT", bufs=18))
            psum1 = ph1.enter_context(tc.tile_pool(name="psum1", bufs=3, space="PSUM"))
            psum1b = ph1.enter_context(
                tc.tile_pool(name="psum1b", bufs=2, space="PSUM")
            )

            # mask and projection-bias vectors ride the gpsimd queue
            # FIRST (tiny; ahead of the 3.5MB of W); loaded as [rows, 128]
            # tiles (contiguous 512B per partition), PE-transposed later.
            mrow_i = const.tile([KT, P], I32, tag="mrow_i")
            nc.gpsimd.dma_start(mrow_i[:], mask_ap.rearrange("(a p) -> a p", p=P))
            mrow = const.tile([KT, P], F32, tag="mrow")
            nc.vector.tensor_copy(mrow[:], mrow_i[:])
            brow = {}
            for wname in ("q", "k"):
                br = const.tile([3, P], F32, tag=f"brow{wname}")
                nc.gpsimd.dma_start(
                    br[:], aps[f"b{wname}"].rearrange("(a p) -> a p", p=P)
                )
                brow[wname] = br
            bv_bc = const.tile([P, NH, HD], F32)
            nc.gpsimd.dma_start(
                bv_bc[:],
                aps["bv"].rearrange("(h d) -> h d", d=HD)[None].to_broadcast(
                    (P, NH, HD)
                ),
            )

            # X tiles [128, 768] -> fp16, casts alternating ACT/DVE
            x16 = []
            for t in range(8):
                xt_ = xload.tile([P, HIN], F32, tag="x")
                nc.sync.dma_start(xt_[:], x_ap[t * P:(t + 1) * P, :])
                x16_t = x16_pool.tile([P, HIN], F16, tag="x16", name=f"x16_{t}")
                if t % 2 == 0:
                    nc.scalar.copy(x16_t[:], xt_[:])
                else:
                    nc.vector.tensor_copy(x16_t[:], xt_[:])
                x16.append(x16_t)

            # W loads (gpsimd queue, parallel with X/rel on sync) + ACT casts
            w16 = {}
            for wname in ("q", "k", "v"):
                w_ap = aps[f"w{wname}"]
                w16s = []
                for d in range(3):
                    wt_ = wload.tile([P, HIN], F32, tag="wload")
                    nc.gpsimd.dma_start(wt_[:], w_ap[d * P:(d + 1) * P, :])
                    w16_t = w16_pool.tile(
                        [P, HIN], F16, tag="w16", name=f"w16{wname}_{d}"
                    )
                    nc.scalar.copy(w16_t[:], wt_[:])
                    w16s.append(w16_t)
                w16[wname] = w16s

            # X^T: 6 fp16 tiles [128, 1024] (h-chunk on partitions);
            # psum -> SBUF copies on the DVE (16-bit, 2x rate)
            xT = []
            for hc in range(6):
                pt = psum1.tile([P, S], F16, tag="xtp")  # 1 bank (fp16)
                for t in range(8):
                    nc.tensor.transpose(
                        pt[:, t * P:(t + 1) * P],
                        x16[t][:, hc * P:(hc + 1) * P],
                        ident[:],
                    )
                xt_t = xt_pool.tile([P, S], F16, tag="xT")
                nc.vector.tensor_copy(xt_t[:], pt[:])
                xT.append(xt_t)

            pm = psum1b.tile([P, 512], F32, tag="projp", name="pm")
            nc.tensor.transpose(pm[:, 0:KT], mrow[:], ident32[:KT, :KT])
            nc.tensor.transpose(pm[:, 16:19], brow["q"][:], ident32[:3, :3])
            nc.tensor.transpose(pm[:, 24:27], brow["k"][:], ident32[:3, :3])
            maskb = const.tile([P, KT], F32)
            nc.vector.tensor_scalar_mul(maskb[:], pm[:, 0:KT], NEG)
            bias_sb = {}
            bq_t = const.tile([P, 3], F32, tag="bq")
            nc.vector.tensor_scalar_mul(bq_t[:], pm[:, 16:19], 0.125)
            bias_sb["q"] = bq_t
            bk_t = const.tile([P, 3], F32, tag="bk")
            nc.vector.tensor_copy(bk_t[:], pm[:, 24:27])
            bias_sb["k"] = bk_t

            # W^T slices (fp16): wT[(w, hc)] = [128, 384] (copies on ACT)
            wT = {}
            for wname in ("q", "k", "v"):
                w16s = w16[wname]
                for hc in range(6):
                    pw = psum1b.tile([P, 512], F16, tag="ps1b", name="pw")[:, :HOUT]
                    for d in range(3):
                        nc.tensor.transpose(
                            pw[:, d * P:(d + 1) * P],
                            w16s[d][:, hc * P:(hc + 1) * P],
                            ident[:],
                        )
                    wt_t = wt_pool.tile([P, HOUT], F16, tag="wT")
                    nc.scalar.copy(wt_t[:], pw[:])
                    wT[(wname, hc)] = wt_t

            # Q^T, K^T projections: fp16 matmuls, fp32 PSUM
            for wname, dest, scale in (("q", qT, 0.125), ("k", kT, 1.0)):
                for d in range(3):
                    for tch in range(2):
                        pp = psum1b.tile([P, 512], F32, tag="projp")
                        for hc in range(6):
                            nc.tensor.matmul(
                                pp[:],
                                wT[(wname, hc)][:, d * P:(d + 1) * P],
                                xT[hc][:, tch * 512:(tch + 1) * 512],
                                start=(hc == 0),
                                stop=(hc == 5),
                            )
                        nc.scalar.activation(
                            dest[d][:, tch * 512:(tch + 1) * 512],
                            pp[:],
                            AF.Identity,
                            bias=bias_sb[wname][:, d:d + 1],
                            scale=scale,
                        )

            # V projection: out [t-tile 128, 384] fp16 + ones column
            for t in range(8):
                pv = psum1b.tile([P, 512], F32, tag="projp", name="pv")[:, :HOUT]
                for hc in range(6):
                    nc.tensor.matmul(
                        pv[:],
                        xT[hc][:, t * P:(t + 1) * P],
                        wT[("v", hc)][:],
                        start=(hc == 0),
                        stop=(hc == 5),
                    )
                nc.vector.memset(v_tiles[t][:], 1.0)
                # copy + bias add (bv broadcast along partitions)
                nc.vector.tensor_add(
                    v_tiles[t][:, :, 0:HD],
                    pv[:].rearrange("p (h d) -> p h d", d=HD),
                    bv_bc[:],
                )

        # ---------------- phase 2: attention per head ----------------
        out_pool = ctx.enter_context(tc.tile_pool(name="outst", bufs=8))
        out_stage = [
            out_pool.tile([P, HOUT], F32, tag="outst", name=f"outst{i}")
            for i in range(8)
        ]
        pt_pool = ctx.enter_context(tc.tile_pool(name="pT", bufs=8))
        fin_pool = ctx.enter_context(tc.tile_pool(name="fin", bufs=8))
        ctt_pool = ctx.enter_context(tc.tile_pool(name="ctt", bufs=4))
        ctsb_pool = ctx.enter_context(tc.tile_pool(name="ctsb", bufs=2))
        spsum = ctx.enter_context(tc.tile_pool(name="spsum", bufs=6, space="PSUM"))
        vpsum = ctx.enter_context(tc.tile_pool(name="vpsum", bufs=2, space="PSUM"))

        def load_strips(h):
            """rel1+rel2 -> fp16 strip tiles for head h (DVE add)."""
            strips = []
            for qq in range(8):
                r1 = r1_pool.tile([P, S], F32, tag="r1")
                nc.sync.dma_start(
                    r1[:],
                    rel1_ap[h].rearrange("(qt p) k -> p qt k", p=P)[:, qq, :],
                )
                r2 = r2_pool.tile([P, S], F32, tag="r2")
                nc.sync.dma_start(
                    r2[:],
                    rel2_ap[h].rearrange("(qt p) k -> p qt k", p=P)[:, qq, :],
                )
                rb = rbf_pool.tile([P, S], F16, tag="rbf", name=f"rbf{h}_{qq}")
                nc.vector.tensor_add(rb[:], r1[:], r2[:])
                strips.append(rb)
            return strips

        def emit_finalize(h, ctxT_ps, last=False):
            """Epilogue for head h: ctx^T out of PSUM as fp16 (DVE copy),
            back-transpose on the PE (fp16, 1 cyc/row) into a dedicated
            1-bank psum, evacuate once to SBUF (DVE), then divide by the
            denominator column reading SBUF only -- the score-psum ring
            and the PE stream never wait on the recip/scale chain.
            Deferred one head so the in-order PE stream never stalls."""
            ctxT_sb = []
            for qch in range(2):
                t16 = ctt_pool.tile([VW, 512], F16, tag="ctxT16",
                                    name=f"ctxTs{h}_{qch}")
                nc.vector.tensor_copy(t16[:], ctxT_ps[qch][:])
                ctxT_sb.append(t16)
            ctx_ps = spsum.tile([P, QT * VS], F16, tag="sT", name=f"ctxp{h}")
            for qt in range(QT):
                sl = qt * VS
                nc.tensor.transpose(
                    ctx_ps[:, sl:sl + VW],
                    ctxT_sb[qt // 4][:, (qt % 4) * P:(qt % 4 + 1) * P],
                    ident[:VW, :VW],
                )
            ctx_sb = ctsb_pool.tile([P, QT * VS], F16, tag="ctsb",
                                    name=f"ctxsb{h}")
            nc.vector.tensor_copy(ctx_sb[:], ctx_ps[:])
            for qt in range(QT):
                sl = qt * VS
                rc = fin_pool.tile([P, 1], F32, tag="recip")
                nc.vector.reciprocal(rc[:], ctx_sb[:, sl + HD:sl + HD + 1])
                nc.scalar.activation(
                    out_stage[qt][:, h * HD:(h + 1) * HD],
                    ctx_sb[:, sl:sl + HD],
                    AF.Identity,
                    scale=rc[:],
                )
                if last:
                    nc.sync.dma_start(
                        out_ap[qt * P:(qt + 1) * P, :], out_stage[qt][:]
                    )

        pending_fin = None
        for h in range(NH):
            strips = load_strips(h)
            dt, rem = divmod(h, 2)
            d0 = rem * HD
            qTh = qT[dt][d0:d0 + HD, :]
            kTh = kT[dt][d0:d0 + HD, :]

            ctxT_ps = [
                vpsum.tile([VW, 512], F32, tag="ctxT", name=f"ctxT{h}_{i}")
                for i in range(2)
            ]

            pT_prev = None
            for kt in range(KT):
                kl = slice(kt * P, (kt + 1) * P)
                pT_cur = pt_pool.tile([P, S], F16, tag="pT", name=f"pT{h}_{kt}")
                for qch in range(2):
                    ps = spsum.tile([P, 512], F32, tag="sT")
                    # qk^T
                    nc.tensor.matmul(
                        ps[:],
                        kTh[:, kl],
                        qTh[:, qch * 512:(qch + 1) * 512],
                        start=True,
                        stop=False,
                    )
                    # += rel12^T (transposing adds via fp16 identity rhs)
                    for j in range(4):
                        qt = qch * 4 + j
                        nc.tensor.matmul(
                            ps[:, j * P:(j + 1) * P],
                            strips[qt][:, kl],
                            ident[:],
                            start=False,
                            stop=(j == 3),
                        )
                    # exp(scores + mask bias) -> fp16 probs
                    nc.scalar.activation(
                        pT_cur[:, qch * 512:(qch + 1) * 512],
                        ps[:],
                        AF.Exp,
                        bias=maskb[:, kt:kt + 1],
                        scale=1.0,
                    )
                # PV for the previous kt (one behind: the PE never waits
                # on the exps of the current kt)
                if pT_prev is not None:
                    for qch in range(2):
                        nc.tensor.matmul(
                            ctxT_ps[qch][:],
                            v_tiles[kt - 1][:, h, :],
                            pT_prev[:, qch * 512:(qch + 1) * 512],
                            start=(kt == 1),
                            stop=False,
                        )
                pT_prev = pT_cur
                if kt == 0 and pending_fin is not None:
                    emit_finalize(*pending_fin)
                    pending_fin = None

            # last kt's PV closes the accumulation
            for qch in range(2):
                nc.tensor.matmul(
                    ctxT_ps[qch][:],
                    v_tiles[KT - 1][:, h, :],
                    pT_prev[:, qch * 512:(qch + 1) * 512],
                    start=False,
                    stop=True,
                )

            pending_fin = (h, ctxT_ps)

        emit_finalize(*pending_fin, last=True)


def build_program():
    """Build and compile the per-core Bass program. Returns nc."""
    nc = bacc.Bacc(
        "TRN2",
        target_bir_lowering=False,
        debug=False,
        num_devices=8,
    )
    aps = {
        "x": nc.dram_tensor("x", [S, HIN], F32, kind="ExternalInput").ap(),
        "mask": nc.dram_tensor("mask", [S], I32, kind="ExternalInput").ap(),
        "rel1": nc.dram_tensor("rel1", [NH, S, S], F32, kind="ExternalInput").ap(),
        "rel2": nc.dram_tensor("rel2", [NH, S, S], F32, kind="ExternalInput").ap(),
        "wq": nc.dram_tensor("wq", [HOUT, HIN], F32, kind="ExternalInput").ap(),
        "wk": nc.dram_tensor("wk", [HOUT, HIN], F32, kind="ExternalInput").ap(),
        "wv": nc.dram_tensor("wv", [HOUT, HIN], F32, kind="ExternalInput").ap(),
        "bq": nc.dram_tensor("bq", [HOUT], F32, kind="ExternalInput").ap(),
        "bk": nc.dram_tensor("bk", [HOUT], F32, kind="ExternalInput").ap(),
        "bv": nc.dram_tensor("bv", [HOUT], F32, kind="ExternalInput").ap(),
        "out": nc.dram_tensor("out", [S, HOUT], F32, kind="ExternalOutput").ap(),
    }
    with tile.TileContext(nc) as tc:
        _build_kernel_body(tc, aps)
    nc.compile()
    return nc


def make_in_maps(inputs):
    """Slice full inputs into the 8 per-core input maps."""
    hs = np.ascontiguousarray(np.asarray(inputs["hidden_states"], np.float32))
    am = np.asarray(inputs["attention_mask"]).astype(np.int32)
    rel1 = np.asarray(inputs["rel_pos"], np.float32)
    rel2 = np.asarray(inputs["rel_2d_pos"], np.float32)
    ws = {k: np.asarray(inputs["W" + k[-1]], np.float32) for k in ("wq", "wk", "wv")}
    bs = {k: np.asarray(inputs["b" + k[-1]], np.float32) for k in ("bq", "bk", "bv")}

    in_maps = []
    for c in range(8):
        b, hh = divmod(c, 2)
        hsl = slice(hh * NH, (hh + 1) * NH)
        csl = slice(hh * HOUT, (hh + 1) * HOUT)
        m = {
            "x": np.ascontiguousarray(hs[b]),
            "mask": np.ascontiguousarray(am[b, 0, 0]),
            "rel1": np.ascontiguousarray(rel1[b, hsl]),
            "rel2": np.ascontiguousarray(rel2[b, hsl]),
        }
        for k in ("wq", "wk", "wv"):
            m[k] = np.ascontiguousarray(ws[k][csl])
        for k in ("bq", "bk", "bv"):
            m[k] = np.ascontiguousarray(bs[k][csl])
        in_maps.append(m)
    return in_maps


def gather_output(results):
    out = np.empty((4, S, HIN), np.float32)
    for c in range(8):
        b, hh = divmod(c, 2)
        out[b, :, hh * HOUT:(hh + 1) * HOUT] = results[c]["out"]
    return out


_NC_CACHE = []


def kernel(**inputs):
    if not _NC_CACHE:
        _NC_CACHE.append(build_program())
    nc = _NC_CACHE[0]
    in_maps = make_in_maps(inputs)
    res = run_bass_kernel_spmd(nc, in_maps, list(range(8)))
    return gather_output(res.results)


# revision 29
# speedup vs baseline: 1.0313x; 1.0032x over previous
"""ErnieLayout self-attention on 8 Trainium2 NeuronCores (Bass/Tile).

Problem shapes (hardcoded): B=4, S=1024, H=768, NH=12, HD=64.
Sharding: core c -> (batch b = c//2, head-half hh = c%2, i.e. 6 heads).
Each core computes attention for its 6 heads of one batch element and
writes the [S, 384] column slice of that batch's output.

v5 design (per-core, mixed precision, scores kept transposed):
  rel12 = rel_pos + rel_2d_pos: fp32 strips stream on the sync (HWDGE)
  queue -- the only DMA path that sustains ~400 GB/s here (SWDGE
  cast/accum DMAs cap at ~150 GB/s write-side and XBAR-transpose DMAs
  serialize against every other in-flight DMA; both were measured on
  HW and rejected).  A DVE pass adds each fp32 pair into fp16 strip
  tiles; r1/r2 staging 8 deep so the strip DMAs run a full head ahead
  of the adds, 28 fp16 strip tiles = 3.5 heads of prefetch.

  setup:  mask/bias vectors ride the gpsimd (SWDGE) queue first, then
          the W tiles -- the sync queue carries only X + rel + out, so
          the rel stream starts ~10us earlier.  X casts to fp16
          alternate ACT/DVE, W casts ride ACT; all transposed on the
          PE (fp16).  Q^T = (Wq_s @ X^T + bq)/8, K^T = Wk_s @ X^T +
          bk (fp16 matmuls, fp32 PSUM accumulate); V = X @ Wv_s^T
          (+ bv), fp16 with a ones column appended (col 64 -> softmax
          denominator for free).  mask/bias are loaded as [rows, 128]
          tiles (contiguous 512B descriptors) and PE-transposed after
          the X^T section -- no 4B-element gather DMAs anywhere and
          nothing early in the PE stream waits on the W queue.
  scores: per (head, ktile, q-chunk):
          psum[k=128, q=512] = K^T.T @ Q^T  (fp16, 1 cyc/row)
          psum += rel12[q,ktile]^T via matmul(lhsT=rel12_f16, rhs=I)
          pT = exp(psum + maskbias) -> fp16 (ACT per-partition bias;
          masked keys get FLT_MIN so exp underflows to exactly 0).
          spsum ring of 6 banks lets the PE run ~1.5 ktiles ahead of
          the exps instead of lock-stepping with ACT.
  PV:     ctx^T[d|1, q-chunk] += V_aug[kt].T @ pT[kt], emitted one kt
          behind the exp that produces pT[kt] (the PE never waits on
          ACT, and the post-last-DMA tail is ~one kt of work).
  fin:    ctx^T -> SBUF fp16 on the DVE (keeps ACT free for exps),
          back-transposed on the PE in fp16 (1 cyc/row; ctx values are
          O(1e3), well inside fp16), evacuated from PSUM to SBUF in one
          DVE copy so the recip (DVE) / scale (ACT) chain never
          ping-pongs on a PSUM bank and the score-psum ring frees
          immediately (this was a measured ~3.7us/head PE stall);
          out[q, h*64+d] = ctx[q, d] * (1 / ctx[q, 64]); the finalize
          of head h is emitted inside head h+1's loop, and the last
          head's output DMAs are interleaved with its scales.
Precision: fp16 carries 10 mantissa bits -> final rel err ~1e-3.

Measured (this exact program, 8-core SPMD, same-day conditions):
  217.7 / 219.5 / 219.8 / 225.0 us, rel err 1.023e-3  (baseline 244.8).
  Run-to-run spread is bimodal +/-4% from a chip-level power/HAM
  throttle that holds the PE at K=4/8 (1.2 GHz) for ~half the steady
  state with all 8 cores active; at full clock the identical stream
  would be DMA-bound at ~170 us (DMA sustains ~336 GB/s, ends ~177 us
  of ~218).  Structural variants measured and rejected (totals in us):
  smooth pacing w/ spsum=5 (268, locks the throttle on), fin copies on
  ACT (220/225 despite best-ever 22.5 us/head cadence -- delays exps),
  hybrid fin split (231), col/row-packed matmul pairs (236/306, no
  concurrency materializes), t-major X^T (220), deferred V-proj
  (218/226).  Emitting the finalize later than ktile 0 deadlocks via
  the 2-slot ctx-psum ring (PE waits a copy whose ACT-queue
  predecessors wait the PE).
"""

import os
import sys

import numpy as np

for _p in ("/opt/trn_rl_repo",):
    if _p not in sys.path and os.path.isdir(_p):
        sys.path.append(_p)

import concourse.bass as bass
import concourse.mybir as mybir
import concourse.tile as tile
from concourse import bacc
from concourse.bass_utils import run_bass_kernel_spmd
from concourse.masks import make_identity

F32 = mybir.dt.float32
F16 = mybir.dt.float16
I32 = mybir.dt.int32
AF = mybir.ActivationFunctionType
NEG = float(np.finfo(np.float32).min)

P = 128
S = 1024
NH = 6        # heads per core
HD = 64
HIN = 768     # model dim (contraction for projections)
HOUT = NH * HD  # 384, per-core projection width
KT = S // P   # 8 key tiles
QT = S // P   # 8 query tiles
VW = HD + 1   # 65: V columns + ones column
VS = VW + 1   # 66: psum stride per transposed block (4B-aligned fp16)


def _build_kernel_body(tc, aps):
    import contextlib

    nc = tc.nc
    x_ap = aps["x"]
    mask_ap = aps["mask"]
    rel1_ap = aps["rel1"]
    rel2_ap = aps["rel2"]
    out_ap = aps["out"]

    with contextlib.ExitStack() as ctx:
        const = ctx.enter_context(tc.tile_pool(name="const", bufs=1))

        ident = const.tile([P, P], F16)
        make_identity(nc, ident)
        ident32 = const.tile([P, P], F32)
        nc.scalar.copy(ident32[:], ident[:])

        # long-lived tensors
        qt_pool = ctx.enter_context(tc.tile_pool(name="qT", bufs=3))
        kt_pool = ctx.enter_context(tc.tile_pool(name="kT", bufs=3))
        v_pool = ctx.enter_context(tc.tile_pool(name="v", bufs=8))

        qT = [qt_pool.tile([P, S], F16, tag="qT", name=f"qT{i}") for i in range(3)]
        kT = [kt_pool.tile([P, S], F16, tag="kT", name=f"kT{i}") for i in range(3)]
        v_tiles = [
            v_pool.tile([P, NH, VW], F16, tag="v", name=f"v{i}") for i in range(8)
        ]

        # rel stream pools first: their SBUF is disjoint from phase-1 pools,
        # so rel DMA + DVE adds run from t=0 and deep fp16 buffering keeps
        # the DMA queues fed across head boundaries.
        r1_pool = ctx.enter_context(tc.tile_pool(name="r1", bufs=8))
        r2_pool = ctx.enter_context(tc.tile_pool(name="r2", bufs=8))
        rbf_pool = ctx.enter_context(tc.tile_pool(name="rbf", bufs=28))

        # ---------------- phase 1: load, cast, transpose, project ----------
        with contextlib.ExitStack() as ph1:
            xload = ph1.enter_context(tc.tile_pool(name="xload", bufs=2))
            wload = ph1.enter_context(tc.tile_pool(name="wload", bufs=2))
            x16_pool = ph1.enter_context(tc.tile_pool(name="x16", bufs=8))
            w16_pool = ph1.enter_context(tc.tile_pool(name="w16", bufs=4))
            xt_pool = ph1.enter_context(tc.tile_pool(name="xT", bufs=6))
            wt_pool = ph1.enter_context(tc.tile_pool(name="wT", bufs=18))
            psum1 = ph1.enter_context(tc.tile_pool(name="psum1", bufs=3, space="PSUM"))
            psum1b = ph1.enter_context(
                tc.tile_pool(name="psum1b", bufs=2, space="PSUM")
            )

            # mask and projection-bias vectors ride the gpsimd queue
            # FIRST (tiny; ahead of the 3.5MB of W); loaded as [rows, 128]
            # tiles (contiguous 512B per partition), PE-transposed later.
            mrow_i = const.tile([KT, P], I32, tag="mrow_i")
            nc.gpsimd.dma_start(mrow_i[:], mask_ap.rearrange("(a p) -> a p", p=P))
            mrow = const.tile([KT, P], F32, tag="mrow")
            nc.vector.tensor_copy(mrow[:], mrow_i[:])
            brow = {}
            for wname in ("q", "k"):
                br = const.tile([3, P], F32, tag=f"brow{wname}")
                nc.gpsimd.dma_start(
                    br[:], aps[f"b{wname}"].rearrange("(a p) -> a p", p=P)
                )
                brow[wname] = br
            bv_bc = const.tile([P, NH, HD], F32)
            nc.gpsimd.dma_start(
                bv_bc[:],
                aps["bv"].rearrange("(h d) -> h d", d=HD)[None].to_broadcast(
                    (P, NH, HD)
                ),
            )

            # X tiles [128, 768] -> fp16, casts alternating ACT/DVE
            x16 = []
            for t in range(8):
                xt_ = xload.tile([P, HIN], F32, tag="x")
                nc.sync.dma_start(xt_[:], x_ap[t * P:(t + 1) * P, :])
                x16_t = x16_pool.tile([P, HIN], F16, tag="x16", name=f"x16_{t}")
                if t % 2 == 0:
                    nc.scalar.copy(x16_t[:], xt_[:])
                else:
                    nc.vector.tensor_copy(x16_t[:], xt_[:])
                x16.append(x16_t)

            # W loads (gpsimd queue, parallel with X/rel on sync) + ACT casts
            w16 = {}
            for wname in ("q", "k", "v"):
                w_ap = aps[f"w{wname}"]
                w16s = []
                for d in range(3):
                    wt_ = wload.tile([P, HIN], F32, tag="wload")
                    nc.gpsimd.dma_start(wt_[:], w_ap[d * P:(d + 1) * P, :])
                    w16_t = w16_pool.tile(
                        [P, HIN], F16, tag="w16", name=f"w16{wname}_{d}"
                    )
                    nc.scalar.copy(w16_t[:], wt_[:])
                    w16s.append(w16_t)
                w16[wname] = w16s

            # X^T: 6 fp16 tiles [128, 1024] (h-chunk on partitions);
            # psum -> SBUF copies on the DVE (16-bit, 2x rate)
            xT = []
            for hc in range(6):
                pt = psum1.tile([P, S], F16, tag="xtp")  # 1 bank (fp16)
                for t in range(8):
                    nc.tensor.transpose(
                        pt[:, t * P:(t + 1) * P],
                        x16[t][:, hc * P:(hc + 1) * P],
                        ident[:],
                    )
                xt_t = xt_pool.tile([P, S], F16, tag="xT")
                nc.vector.tensor_copy(xt_t[:], pt[:])
                xT.append(xt_t)

            pm = psum1b.tile([P, 512], F32, tag="projp", name="pm")
            nc.tensor.transpose(pm[:, 0:KT], mrow[:], ident32[:KT, :KT])
            nc.tensor.transpose(pm[:, 16:19], brow["q"][:], ident32[:3, :3])
            nc.tensor.transpose(pm[:, 24:27], brow["k"][:], ident32[:3, :3])
            maskb = const.tile([P, KT], F32)
            nc.vector.tensor_scalar_mul(maskb[:], pm[:, 0:KT], NEG)
            bias_sb = {}
            bq_t = const.tile([P, 3], F32, tag="bq")
            nc.vector.tensor_scalar_mul(bq_t[:], pm[:, 16:19], 0.125)
            bias_sb["q"] = bq_t
            bk_t = const.tile([P, 3], F32, tag="bk")
            nc.vector.tensor_copy(bk_t[:], pm[:, 24:27])
            bias_sb["k"] = bk_t

            # W^T slices (fp16): wT[(w, hc)] = [128, 384] (copies on ACT)
            wT = {}
            for wname in ("q", "k", "v"):
                w16s = w16[wname]
                for hc in range(6):
                    pw = psum1b.tile([P, 512], F16, tag="ps1b", name="pw")[:, :HOUT]
                    for d in range(3):
                        nc.tensor.transpose(
                            pw[:, d * P:(d + 1) * P],
                            w16s[d][:, hc * P:(hc + 1) * P],
                            ident[:],
                        )
                    wt_t = wt_pool.tile([P, HOUT], F16, tag="wT")
                    nc.scalar.copy(wt_t[:], pw[:])
                    wT[(wname, hc)] = wt_t

            # Q^T, K^T projections: fp16 matmuls, fp32 PSUM
            for wname, dest, scale in (("q", qT, 0.125), ("k", kT, 1.0)):
                for d in range(3):
                    for tch in range(2):
                        pp = psum1b.tile([P, 512], F32, tag="projp")
                        for hc in range(6):
                            nc.tensor.matmul(
                                pp[:],
                                wT[(wname, hc)][:, d * P:(d + 1) * P],
                                xT[hc][:, tch * 512:(tch + 1) * 512],
                                start=(hc == 0),
                                stop=(hc == 5),
                            )
                        nc.scalar.activation(
                            dest[d][:, tch * 512:(tch + 1) * 512],
                            pp[:],
                            AF.Identity,
                            bias=bias_sb[wname][:, d:d + 1],
                            scale=scale,
                        )

            # V projection: out [t-tile 128, 384] fp16 + ones column
            for t in range(8):
                pv = psum1b.tile([P, 512], F32, tag="projp", name="pv")[:, :HOUT]
                for hc in range(6):
                    nc.tensor.matmul(
                        pv[:],
                        xT[hc][:, t * P:(t + 1) * P],
                        wT[("v", hc)][:],
                        start=(hc == 0),
                        stop=(hc == 5),
                    )
                nc.vector.memset(v_tiles[t][:], 1.0)
                # copy + bias add (bv broadcast along partitions)
                nc.vector.tensor_add(
                    v_tiles[t][:, :, 0:HD],
                    pv[:].rearrange("p (h d) -> p h d", d=HD),
                    bv_bc[:],
                )

        # ---------------- phase 2: attention per head ----------------
        out_pool = ctx.enter_context(tc.tile_pool(name="outst", bufs=8))
        out_stage = [
            out_pool.tile([P, HOUT], F32, tag="outst", name=f"outst{i}")
            for i in range(8)
        ]
        pt_pool = ctx.enter_context(tc.tile_pool(name="pT", bufs=8))
        fin_pool = ctx.enter_context(tc.tile_pool(name="fin", bufs=8))
        ctt_pool = ctx.enter_context(tc.tile_pool(name="ctt", bufs=4))
        ctsb_pool = ctx.enter_context(tc.tile_pool(name="ctsb", bufs=2))
        spsum = ctx.enter_context(tc.tile_pool(name="spsum", bufs=6, space="PSUM"))
        vpsum = ctx.enter_context(tc.tile_pool(name="vpsum", bufs=2, space="PSUM"))

        def load_strips(h):
            """rel1+rel2 -> fp16 strip tiles for head h (DVE add)."""
            strips = []
            for qq in range(8):
                r1 = r1_pool.tile([P, S], F32, tag="r1")
                nc.sync.dma_start(
                    r1[:],
                    rel1_ap[h].rearrange("(qt p) k -> p qt k", p=P)[:, qq, :],
                )
                r2 = r2_pool.tile([P, S], F32, tag="r2")
                nc.sync.dma_start(
                    r2[:],
                    rel2_ap[h].rearrange("(qt p) k -> p qt k", p=P)[:, qq, :],
                )
                rb = rbf_pool.tile([P, S], F16, tag="rbf", name=f"rbf{h}_{qq}")
                nc.vector.tensor_add(rb[:], r1[:], r2[:])
                strips.append(rb)
            return strips

        def emit_finalize(h, ctxT_ps, last=False, defer=False):
            """Epilogue for head h: ctx^T out of PSUM as fp16 (DVE copy),
            back-transpose on the PE (fp16, 1 cyc/row) into a dedicated
            1-bank psum, evacuate once to SBUF (DVE), then divide by the
            denominator column reading SBUF only -- the score-psum ring
            and the PE stream never wait on the recip/scale chain.
            Deferred one head so the in-order PE stream never stalls."""
            ctxT_sb = []
            for qch in range(2):
                t16 = ctt_pool.tile([VW, 512], F16, tag="ctxT16",
                                    name=f"ctxTs{h}_{qch}")
                nc.vector.tensor_copy(t16[:], ctxT_ps[qch][:])
                ctxT_sb.append(t16)
            ctx_ps = spsum.tile([P, QT * VS], F16, tag="sT", name=f"ctxp{h}")
            for qt in range(QT):
                sl = qt * VS
                nc.tensor.transpose(
                    ctx_ps[:, sl:sl + VW],
                    ctxT_sb[qt // 4][:, (qt % 4) * P:(qt % 4 + 1) * P],
                    ident[:VW, :VW],
                )
            ctx_sb = ctsb_pool.tile([P, QT * VS], F16, tag="ctsb",
                                    name=f"ctxsb{h}")
            nc.vector.tensor_copy(ctx_sb[:], ctx_ps[:])
            def scales(qts):
                for qt in qts:
                    sl = qt * VS
                    rc = fin_pool.tile([P, 1], F32, tag="recip")
                    nc.vector.reciprocal(rc[:], ctx_sb[:, sl + HD:sl + HD + 1])
                    nc.scalar.activation(
                        out_stage[qt][:, h * HD:(h + 1) * HD],
                        ctx_sb[:, sl:sl + HD],
                        AF.Identity,
                        scale=rc[:],
                    )
                    if last:
                        nc.sync.dma_start(
                            out_ap[qt * P:(qt + 1) * P, :], out_stage[qt][:]
                        )

            if defer:
                # half the ACT scales at ktile 0, half at ktile 1: the
                # exp stream never falls a full finalize behind
                scales(range(0, 4))
                return lambda: scales(range(4, QT))
            scales(range(QT))
            return None

        pending_fin = None
        pending_tail = None
        for h in range(NH):
            strips = load_strips(h)
            dt, rem = divmod(h, 2)
            d0 = rem * HD
            qTh = qT[dt][d0:d0 + HD, :]
            kTh = kT[dt][d0:d0 + HD, :]

            ctxT_ps = [
                vpsum.tile([VW, 512], F32, tag="ctxT", name=f"ctxT{h}_{i}")
                for i in range(2)
            ]

            pT_prev = None
            for kt in range(KT):
                kl = slice(kt * P, (kt + 1) * P)
                pT_cur = pt_pool.tile([P, S], F16, tag="pT", name=f"pT{h}_{kt}")
                for qch in range(2):
                    ps = spsum.tile([P, 512], F32, tag="sT")
                    # qk^T
                    nc.tensor.matmul(
                        ps[:],
                        kTh[:, kl],
                        qTh[:, qch * 512:(qch + 1) * 512],
                        start=True,
                        stop=False,
                    )
                    # += rel12^T (transposing adds via fp16 identity rhs)
                    for j in range(4):
                        qt = qch * 4 + j
                        nc.tensor.matmul(
                            ps[:, j * P:(j + 1) * P],
                            strips[qt][:, kl],
                            ident[:],
                            start=False,
                            stop=(j == 3),
                        )
                    # exp(scores + mask bias) -> fp16 probs
                    nc.scalar.activation(
                        pT_cur[:, qch * 512:(qch + 1) * 512],
                        ps[:],
                        AF.Exp,
                        bias=maskb[:, kt:kt + 1],
                        scale=1.0,
                    )
                # PV for the previous kt (one behind: the PE never waits
                # on the exps of the current kt)
                if pT_prev is not None:
                    for qch in range(2):
                        nc.tensor.matmul(
                            ctxT_ps[qch][:],
                            v_tiles[kt - 1][:, h, :],
                            pT_prev[:, qch * 512:(qch + 1) * 512],
                            start=(kt == 1),
                            stop=False,
                        )
                pT_prev = pT_cur
                if kt == 0 and pending_fin is not None:
                    pending_tail = emit_finalize(*pending_fin, defer=True)
                    pending_fin = None
                elif kt == 1 and pending_tail is not None:
                    pending_tail()
                    pending_tail = None

            # last kt's PV closes the accumulation
            for qch in range(2):
                nc.tensor.matmul(
                    ctxT_ps[qch][:],
                    v_tiles[KT - 1][:, h, :],
                    pT_prev[:, qch * 512:(qch + 1) * 512],
                    start=False,
                    stop=True,
                )

            pending_fin = (h, ctxT_ps)

        emit_finalize(*pending_fin, last=True)


def build_program():
    """Build and compile the per-core Bass program. Returns nc."""
    nc = bacc.Bacc(
        "TRN2",
        target_bir_lowering=False,
        debug=False,
        num_devices=8,
    )
    aps = {
        "x": nc.dram_tensor("x", [S, HIN], F32, kind="ExternalInput").ap(),
        "mask": nc.dram_tensor("mask", [S], I32, kind="ExternalInput").ap(),
        "rel1": nc.dram_tensor("rel1", [NH, S, S], F32, kind="ExternalInput").ap(),
        "rel2": nc.dram_tensor("rel2", [NH, S, S], F32, kind="ExternalInput").ap(),
        "wq": nc.dram_tensor("wq", [HOUT, HIN], F32, kind="ExternalInput").ap(),
        "wk": nc.dram_tensor("wk", [HOUT, HIN], F32, kind="ExternalInput").ap(),
        "wv": nc.dram_tensor("wv", [HOUT, HIN], F32, kind="ExternalInput").ap(),
        "bq": nc.dram_tensor("bq", [HOUT], F32, kind="ExternalInput").ap(),
        "bk": nc.dram_tensor("bk", [HOUT], F32, kind="ExternalInput").ap(),
        "bv": nc.dram_tensor("bv", [HOUT], F32, kind="ExternalInput").ap(),
        "out": nc.dram_tensor("out", [S, HOUT], F32, kind="ExternalOutput").ap(),
    }
    with tile.TileContext(nc) as tc:
        _build_kernel_body(tc, aps)
    nc.compile()
    return nc


def make_in_maps(inputs):
    """Slice full inputs into the 8 per-core input maps."""
    hs = np.ascontiguousarray(np.asarray(inputs["hidden_states"], np.float32))
    am = np.asarray(inputs["attention_mask"]).astype(np.int32)
    rel1 = np.asarray(inputs["rel_pos"], np.float32)
    rel2 = np.asarray(inputs["rel_2d_pos"], np.float32)
    ws = {k: np.asarray(inputs["W" + k[-1]], np.float32) for k in ("wq", "wk", "wv")}
    bs = {k: np.asarray(inputs["b" + k[-1]], np.float32) for k in ("bq", "bk", "bv")}

    in_maps = []
    for c in range(8):
        b, hh = divmod(c, 2)
        hsl = slice(hh * NH, (hh + 1) * NH)
        csl = slice(hh * HOUT, (hh + 1) * HOUT)
        m = {
            "x": np.ascontiguousarray(hs[b]),
            "mask": np.ascontiguousarray(am[b, 0, 0]),
            "rel1": np.ascontiguousarray(rel1[b, hsl]),
            "rel2": np.ascontiguousarray(rel2[b, hsl]),
        }
        for k in ("wq", "wk", "wv"):
            m[k] = np.ascontiguousarray(ws[k][csl])
        for k in ("bq", "bk", "bv"):
            m[k] = np.ascontiguousarray(bs[k][csl])
        in_maps.append(m)
    return in_maps


def gather_output(results):
    out = np.empty((4, S, HIN), np.float32)
    for c in range(8):
        b, hh = divmod(c, 2)
        out[b, :, hh * HOUT:(hh + 1) * HOUT] = results[c]["out"]
    return out


_NC_CACHE = []


def kernel(**inputs):
    if not _NC_CACHE:
        _NC_CACHE.append(build_program())
    nc = _NC_CACHE[0]
    in_maps = make_in_maps(inputs)
    res = run_bass_kernel_spmd(nc, in_maps, list(range(8)))
    return gather_output(res.results)


# revision 30
# speedup vs baseline: 1.0417x; 1.0100x over previous
"""ErnieLayout self-attention on 8 Trainium2 NeuronCores (Bass/Tile).

Problem shapes (hardcoded): B=4, S=1024, H=768, NH=12, HD=64.
Sharding: core c -> (batch b = c//2, head-half hh = c%2, i.e. 6 heads).
Each core computes attention for its 6 heads of one batch element and
writes the [S, 384] column slice of that batch's output.

v5 design (per-core, mixed precision, scores kept transposed):
  rel12 = rel_pos + rel_2d_pos: fp32 strips stream on the sync (HWDGE)
  queue -- the only DMA path that sustains ~400 GB/s here (SWDGE
  cast/accum DMAs cap at ~150 GB/s write-side and XBAR-transpose DMAs
  serialize against every other in-flight DMA; both were measured on
  HW and rejected).  A DVE pass adds each fp32 pair into fp16 strip
  tiles; r1/r2 staging 8 deep so the strip DMAs run a full head ahead
  of the adds, 28 fp16 strip tiles = 3.5 heads of prefetch.

  setup:  mask/bias vectors ride the gpsimd (SWDGE) queue first, then
          the W tiles -- the sync queue carries only X + rel + out, so
          the rel stream starts ~10us earlier.  X casts to fp16
          alternate ACT/DVE, W casts ride ACT; all transposed on the
          PE (fp16).  Q^T = (Wq_s @ X^T + bq)/8, K^T = Wk_s @ X^T +
          bk (fp16 matmuls, fp32 PSUM accumulate); V = X @ Wv_s^T
          (+ bv), fp16 with a ones column appended (col 64 -> softmax
          denominator for free).  mask/bias are loaded as [rows, 128]
          tiles (contiguous 512B descriptors) and PE-transposed after
          the X^T section -- no 4B-element gather DMAs anywhere and
          nothing early in the PE stream waits on the W queue.
  scores: per (head, ktile, q-chunk):
          psum[k=128, q=512] = K^T.T @ Q^T  (fp16, 1 cyc/row)
          psum += rel12[q,ktile]^T via matmul(lhsT=rel12_f16, rhs=I)
          pT = exp(psum + maskbias) -> fp16 (ACT per-partition bias;
          masked keys get FLT_MIN so exp underflows to exactly 0).
          spsum ring of 6 banks lets the PE run ~1.5 ktiles ahead of
          the exps instead of lock-stepping with ACT.
  PV:     ctx^T[d|1, q-chunk] += V_aug[kt].T @ pT[kt], emitted one kt
          behind the exp that produces pT[kt] (the PE never waits on
          ACT, and the post-last-DMA tail is ~one kt of work).
  fin:    ctx^T -> SBUF fp16 on the DVE (keeps ACT free for exps),
          back-transposed on the PE in fp16 (1 cyc/row; ctx values are
          O(1e3), well inside fp16), evacuated from PSUM to SBUF in one
          DVE copy so the recip (DVE) / scale (ACT) chain never
          ping-pongs on a PSUM bank and the score-psum ring frees
          immediately (this was a measured ~3.7us/head PE stall);
          out[q, h*64+d] = ctx[q, d] * (1 / ctx[q, 64]); the finalize
          of head h is emitted inside head h+1's loop, and the last
          head's output DMAs are interleaved with its scales.
Precision: fp16 carries 10 mantissa bits -> final rel err ~1e-3.

Measured (this exact program, 8-core SPMD, same-day conditions):
  217.7 / 219.5 / 219.8 / 225.0 us, rel err 1.023e-3  (baseline 244.8).
  Run-to-run spread is bimodal +/-4% from a chip-level power/HAM
  throttle that holds the PE at K=4/8 (1.2 GHz) for ~half the steady
  state with all 8 cores active; at full clock the identical stream
  would be DMA-bound at ~170 us (DMA sustains ~336 GB/s, ends ~177 us
  of ~218).  Structural variants measured and rejected (totals in us):
  smooth pacing w/ spsum=5 (268, locks the throttle on), fin copies on
  ACT (220/225 despite best-ever 22.5 us/head cadence -- delays exps),
  hybrid fin split (231), col/row-packed matmul pairs (236/306, no
  concurrency materializes), t-major X^T (220), deferred V-proj
  (218/226).  Emitting the finalize later than ktile 0 deadlocks via
  the 2-slot ctx-psum ring (PE waits a copy whose ACT-queue
  predecessors wait the PE).
"""

import os
import sys

import numpy as np

for _p in ("/opt/trn_rl_repo",):
    if _p not in sys.path and os.path.isdir(_p):
        sys.path.append(_p)

import concourse.bass as bass
import concourse.mybir as mybir
import concourse.tile as tile
from concourse import bacc
from concourse.bass_utils import run_bass_kernel_spmd
from concourse.masks import make_identity

F32 = mybir.dt.float32
F16 = mybir.dt.float16
I32 = mybir.dt.int32
AF = mybir.ActivationFunctionType
NEG = float(np.finfo(np.float32).min)

P = 128
S = 1024
NH = 6        # heads per core
HD = 64
HIN = 768     # model dim (contraction for projections)
HOUT = NH * HD  # 384, per-core projection width
KT = S // P   # 8 key tiles
QT = S // P   # 8 query tiles
VW = HD + 1   # 65: V columns + ones column
VS = VW + 1   # 66: psum stride per transposed block (4B-aligned fp16)


def _build_kernel_body(tc, aps):
    import contextlib

    nc = tc.nc
    x_ap = aps["x"]
    mask_ap = aps["mask"]
    rel1_ap = aps["rel1"]
    rel2_ap = aps["rel2"]
    out_ap = aps["out"]

    with contextlib.ExitStack() as ctx:
        const = ctx.enter_context(tc.tile_pool(name="const", bufs=1))

        ident = const.tile([P, P], F16)
        make_identity(nc, ident)
        ident32 = const.tile([P, P], F32)
        nc.scalar.copy(ident32[:], ident[:])

        # long-lived tensors
        qt_pool = ctx.enter_context(tc.tile_pool(name="qT", bufs=3))
        kt_pool = ctx.enter_context(tc.tile_pool(name="kT", bufs=3))
        v_pool = ctx.enter_context(tc.tile_pool(name="v", bufs=8))

        qT = [qt_pool.tile([P, S], F16, tag="qT", name=f"qT{i}") for i in range(3)]
        kT = [kt_pool.tile([P, S], F16, tag="kT", name=f"kT{i}") for i in range(3)]
        v_tiles = [
            v_pool.tile([P, NH, VW], F16, tag="v", name=f"v{i}") for i in range(8)
        ]

        # rel stream pools first: their SBUF is disjoint from phase-1 pools,
        # so rel DMA + DVE adds run from t=0 and deep fp16 buffering keeps
        # the DMA queues fed across head boundaries.
        r1_pool = ctx.enter_context(tc.tile_pool(name="r1", bufs=8))
        r2_pool = ctx.enter_context(tc.tile_pool(name="r2", bufs=8))
        rbf_pool = ctx.enter_context(tc.tile_pool(name="rbf", bufs=28))

        # ---------------- phase 1: load, cast, transpose, project ----------
        with contextlib.ExitStack() as ph1:
            xload = ph1.enter_context(tc.tile_pool(name="xload", bufs=2))
            wload = ph1.enter_context(tc.tile_pool(name="wload", bufs=2))
            x16_pool = ph1.enter_context(tc.tile_pool(name="x16", bufs=8))
            w16_pool = ph1.enter_context(tc.tile_pool(name="w16", bufs=4))
            xt_pool = ph1.enter_context(tc.tile_pool(name="xT", bufs=6))
            wt_pool = ph1.enter_context(tc.tile_pool(name="wT", bufs=18))
            psum1 = ph1.enter_context(tc.tile_pool(name="psum1", bufs=3, space="PSUM"))
            psum1b = ph1.enter_context(
                tc.tile_pool(name="psum1b", bufs=2, space="PSUM")
            )

            # mask and projection-bias vectors ride the gpsimd queue
            # FIRST (tiny; ahead of the 3.5MB of W); loaded as [rows, 128]
            # tiles (contiguous 512B per partition), PE-transposed later.
            mrow_i = const.tile([KT, P], I32, tag="mrow_i")
            nc.gpsimd.dma_start(mrow_i[:], mask_ap.rearrange("(a p) -> a p", p=P))
            mrow = const.tile([KT, P], F32, tag="mrow")
            nc.vector.tensor_copy(mrow[:], mrow_i[:])
            brow = {}
            for wname in ("q", "k"):
                br = const.tile([3, P], F32, tag=f"brow{wname}")
                nc.gpsimd.dma_start(
                    br[:], aps[f"b{wname}"].rearrange("(a p) -> a p", p=P)
                )
                brow[wname] = br
            bv_bc = const.tile([P, NH, HD], F32)
            nc.gpsimd.dma_start(
                bv_bc[:],
                aps["bv"].rearrange("(h d) -> h d", d=HD)[None].to_broadcast(
                    (P, NH, HD)
                ),
            )

            # X tiles [128, 768] -> fp16, casts alternating ACT/DVE
            x16 = []
            for t in range(8):
                xt_ = xload.tile([P, HIN], F32, tag="x")
                nc.sync.dma_start(xt_[:], x_ap[t * P:(t + 1) * P, :])
                x16_t = x16_pool.tile([P, HIN], F16, tag="x16", name=f"x16_{t}")
                if t % 2 == 0:
                    nc.scalar.copy(x16_t[:], xt_[:])
                else:
                    nc.vector.tensor_copy(x16_t[:], xt_[:])
                x16.append(x16_t)

            # W loads (gpsimd queue, parallel with X/rel on sync) + ACT casts
            w16 = {}
            for wname in ("q", "k", "v"):
                w_ap = aps[f"w{wname}"]
                w16s = []
                for d in range(3):
                    wt_ = wload.tile([P, HIN], F32, tag="wload")
                    nc.gpsimd.dma_start(wt_[:], w_ap[d * P:(d + 1) * P, :])
                    w16_t = w16_pool.tile(
                        [P, HIN], F16, tag="w16", name=f"w16{wname}_{d}"
                    )
                    nc.scalar.copy(w16_t[:], wt_[:])
                    w16s.append(w16_t)
                w16[wname] = w16s

            # X^T: 6 fp16 tiles [128, 1024] (h-chunk on partitions);
            # psum -> SBUF copies on the DVE (16-bit, 2x rate)
            xT = []
            for hc in range(6):
                pt = psum1.tile([P, S], F16, tag="xtp")  # 1 bank (fp16)
                for t in range(8):
                    nc.tensor.transpose(
                        pt[:, t * P:(t + 1) * P],
                        x16[t][:, hc * P:(hc + 1) * P],
                        ident[:],
                    )
                xt_t = xt_pool.tile([P, S], F16, tag="xT")
                nc.vector.tensor_copy(xt_t[:], pt[:])
                xT.append(xt_t)

            pm = psum1b.tile([P, 512], F32, tag="projp", name="pm")
            nc.tensor.transpose(pm[:, 0:KT], mrow[:], ident32[:KT, :KT])
            nc.tensor.transpose(pm[:, 16:19], brow["q"][:], ident32[:3, :3])
            nc.tensor.transpose(pm[:, 24:27], brow["k"][:], ident32[:3, :3])
            maskb = const.tile([P, KT], F32)
            nc.vector.tensor_scalar_mul(maskb[:], pm[:, 0:KT], NEG)
            bias_sb = {}
            bq_t = const.tile([P, 3], F32, tag="bq")
            nc.vector.tensor_scalar_mul(bq_t[:], pm[:, 16:19], 0.125)
            bias_sb["q"] = bq_t
            bk_t = const.tile([P, 3], F32, tag="bk")
            nc.vector.tensor_copy(bk_t[:], pm[:, 24:27])
            bias_sb["k"] = bk_t

            # W^T slices (fp16): wT[(w, hc)] = [128, 384] (copies on ACT)
            wT = {}
            for wname in ("q", "k", "v"):
                w16s = w16[wname]
                for hc in range(6):
                    pw = psum1b.tile([P, 512], F16, tag="ps1b", name="pw")[:, :HOUT]
                    for d in range(3):
                        nc.tensor.transpose(
                            pw[:, d * P:(d + 1) * P],
                            w16s[d][:, hc * P:(hc + 1) * P],
                            ident[:],
                        )
                    wt_t = wt_pool.tile([P, HOUT], F16, tag="wT")
                    nc.scalar.copy(wt_t[:], pw[:])
                    wT[(wname, hc)] = wt_t

            # Q^T, K^T projections: fp16 matmuls, fp32 PSUM
            for wname, dest, scale in (("q", qT, 0.125), ("k", kT, 1.0)):
                for d in range(3):
                    for tch in range(2):
                        pp = psum1b.tile([P, 512], F32, tag="projp")
                        for hc in range(6):
                            nc.tensor.matmul(
                                pp[:],
                                wT[(wname, hc)][:, d * P:(d + 1) * P],
                                xT[hc][:, tch * 512:(tch + 1) * 512],
                                start=(hc == 0),
                                stop=(hc == 5),
                            )
                        nc.scalar.activation(
                            dest[d][:, tch * 512:(tch + 1) * 512],
                            pp[:],
                            AF.Identity,
                            bias=bias_sb[wname][:, d:d + 1],
                            scale=scale,
                        )

            # V projection: out [t-tile 128, 384] fp16 + ones column
            for t in range(8):
                pv = psum1b.tile([P, 512], F32, tag="projp", name="pv")[:, :HOUT]
                for hc in range(6):
                    nc.tensor.matmul(
                        pv[:],
                        xT[hc][:, t * P:(t + 1) * P],
                        wT[("v", hc)][:],
                        start=(hc == 0),
                        stop=(hc == 5),
                    )
                nc.vector.memset(v_tiles[t][:], 1.0)
                # copy + bias add (bv broadcast along partitions)
                nc.vector.tensor_add(
                    v_tiles[t][:, :, 0:HD],
                    pv[:].rearrange("p (h d) -> p h d", d=HD),
                    bv_bc[:],
                )

        # ---------------- phase 2: attention per head ----------------
        out_pool = ctx.enter_context(tc.tile_pool(name="outst", bufs=8))
        out_stage = [
            out_pool.tile([P, HOUT], F32, tag="outst", name=f"outst{i}")
            for i in range(8)
        ]
        pt_pool = ctx.enter_context(tc.tile_pool(name="pT", bufs=8))
        fin_pool = ctx.enter_context(tc.tile_pool(name="fin", bufs=8))
        ctt_pool = ctx.enter_context(tc.tile_pool(name="ctt", bufs=4))
        ctsb_pool = ctx.enter_context(tc.tile_pool(name="ctsb", bufs=2))
        spsum = ctx.enter_context(tc.tile_pool(name="spsum", bufs=6, space="PSUM"))
        vpsum = ctx.enter_context(tc.tile_pool(name="vpsum", bufs=2, space="PSUM"))

        def load_strips(h):
            """rel1+rel2 -> fp16 strip tiles for head h (DVE add)."""
            strips = []
            for qq in range(8):
                r1 = r1_pool.tile([P, S], F32, tag="r1")
                nc.sync.dma_start(
                    r1[:],
                    rel1_ap[h].rearrange("(qt p) k -> p qt k", p=P)[:, qq, :],
                )
                r2 = r2_pool.tile([P, S], F32, tag="r2")
                nc.sync.dma_start(
                    r2[:],
                    rel2_ap[h].rearrange("(qt p) k -> p qt k", p=P)[:, qq, :],
                )
                rb = rbf_pool.tile([P, S], F16, tag="rbf", name=f"rbf{h}_{qq}")
                nc.vector.tensor_add(rb[:], r1[:], r2[:])
                strips.append(rb)
            return strips

        def emit_finalize(h, ctxT_ps, last=False):
            """Epilogue for head h: ctx^T out of PSUM as fp16 (DVE copy),
            back-transpose on the PE (fp16, 1 cyc/row) into a dedicated
            1-bank psum, evacuate once to SBUF (DVE), then divide by the
            denominator column reading SBUF only -- the score-psum ring
            and the PE stream never wait on the recip/scale chain.
            Deferred one head so the in-order PE stream never stalls."""
            ctxT_sb = []
            for qch in range(2):
                t16 = ctt_pool.tile([VW, 512], F16, tag="ctxT16",
                                    name=f"ctxTs{h}_{qch}")
                nc.vector.tensor_copy(t16[:], ctxT_ps[qch][:])
                ctxT_sb.append(t16)
            ctx_ps = spsum.tile([P, QT * VS], F16, tag="sT", name=f"ctxp{h}")
            for qt in range(QT):
                sl = qt * VS
                nc.tensor.transpose(
                    ctx_ps[:, sl:sl + VW],
                    ctxT_sb[qt // 4][:, (qt % 4) * P:(qt % 4 + 1) * P],
                    ident[:VW, :VW],
                )
            ctx_sb = ctsb_pool.tile([P, QT * VS], F16, tag="ctsb",
                                    name=f"ctxsb{h}")
            nc.vector.tensor_copy(ctx_sb[:], ctx_ps[:])
            for qt in range(QT):
                sl = qt * VS
                rc = fin_pool.tile([P, 1], F32, tag="recip")
                nc.vector.reciprocal(rc[:], ctx_sb[:, sl + HD:sl + HD + 1])
                nc.scalar.activation(
                    out_stage[qt][:, h * HD:(h + 1) * HD],
                    ctx_sb[:, sl:sl + HD],
                    AF.Identity,
                    scale=rc[:],
                )
                if last:
                    nc.sync.dma_start(
                        out_ap[qt * P:(qt + 1) * P, :], out_stage[qt][:]
                    )

        pending_fin = None
        for h in range(NH):
            strips = load_strips(h)
            dt, rem = divmod(h, 2)
            d0 = rem * HD
            qTh = qT[dt][d0:d0 + HD, :]
            kTh = kT[dt][d0:d0 + HD, :]

            ctxT_ps = [
                vpsum.tile([VW, 512], F32, tag="ctxT", name=f"ctxT{h}_{i}")
                for i in range(2)
            ]

            pT_prev = None
            for kt in range(KT):
                kl = slice(kt * P, (kt + 1) * P)
                pT_cur = pt_pool.tile([P, S], F16, tag="pT", name=f"pT{h}_{kt}")
                for qch in range(2):
                    ps = spsum.tile([P, 512], F32, tag="sT")
                    # qk^T
                    nc.tensor.matmul(
                        ps[:],
                        kTh[:, kl],
                        qTh[:, qch * 512:(qch + 1) * 512],
                        start=True,
                        stop=False,
                    )
                    # += rel12^T (transposing adds via fp16 identity rhs)
                    for j in range(4):
                        qt = qch * 4 + j
                        nc.tensor.matmul(
                            ps[:, j * P:(j + 1) * P],
                            strips[qt][:, kl],
                            ident[:],
                            start=False,
                            stop=(j == 3),
                        )
                    # exp(scores + mask bias) -> fp16 probs
                    nc.scalar.activation(
                        pT_cur[:, qch * 512:(qch + 1) * 512],
                        ps[:],
                        AF.Exp,
                        bias=maskb[:, kt:kt + 1],
                        scale=1.0,
                    )
                # PV for the previous kt (one behind: the PE never waits
                # on the exps of the current kt)
                if pT_prev is not None:
                    for qch in range(2):
                        nc.tensor.matmul(
                            ctxT_ps[qch][:],
                            v_tiles[kt - 1][:, h, :],
                            pT_prev[:, qch * 512:(qch + 1) * 512],
                            start=(kt == 1),
                            stop=False,
                        )
                pT_prev = pT_cur
                if kt == 0 and pending_fin is not None:
                    emit_finalize(*pending_fin)
                    pending_fin = None

            # last kt's PV closes the accumulation
            for qch in range(2):
                nc.tensor.matmul(
                    ctxT_ps[qch][:],
                    v_tiles[KT - 1][:, h, :],
                    pT_prev[:, qch * 512:(qch + 1) * 512],
                    start=False,
                    stop=True,
                )

            pending_fin = (h, ctxT_ps)

        emit_finalize(*pending_fin, last=True)


def build_program():
    """Build and compile the per-core Bass program. Returns nc."""
    nc = bacc.Bacc(
        "TRN2",
        target_bir_lowering=False,
        debug=False,
        num_devices=8,
    )
    aps = {
        "x": nc.dram_tensor("x", [S, HIN], F32, kind="ExternalInput").ap(),
        "mask": nc.dram_tensor("mask", [S], I32, kind="ExternalInput").ap(),
        "rel1": nc.dram_tensor("rel1", [NH, S, S], F32, kind="ExternalInput").ap(),
        "rel2": nc.dram_tensor("rel2", [NH, S, S], F32, kind="ExternalInput").ap(),
        "wq": nc.dram_tensor("wq", [HOUT, HIN], F32, kind="ExternalInput").ap(),
        "wk": nc.dram_tensor("wk", [HOUT, HIN], F32, kind="ExternalInput").ap(),
        "wv": nc.dram_tensor("wv", [HOUT, HIN], F32, kind="ExternalInput").ap(),
        "bq": nc.dram_tensor("bq", [HOUT], F32, kind="ExternalInput").ap(),
        "bk": nc.dram_tensor("bk", [HOUT], F32, kind="ExternalInput").ap(),
        "bv": nc.dram_tensor("bv", [HOUT], F32, kind="ExternalInput").ap(),
        "out": nc.dram_tensor("out", [S, HOUT], F32, kind="ExternalOutput").ap(),
    }
    with tile.TileContext(nc) as tc:
        _build_kernel_body(tc, aps)
    nc.compile()
    return nc


def make_in_maps(inputs):
    """Slice full inputs into the 8 per-core input maps."""
    hs = np.ascontiguousarray(np.asarray(inputs["hidden_states"], np.float32))
    am = np.asarray(inputs["attention_mask"]).astype(np.int32)
    rel1 = np.asarray(inputs["rel_pos"], np.float32)
    rel2 = np.asarray(inputs["rel_2d_pos"], np.float32)
    ws = {k: np.asarray(inputs["W" + k[-1]], np.float32) for k in ("wq", "wk", "wv")}
    bs = {k: np.asarray(inputs["b" + k[-1]], np.float32) for k in ("bq", "bk", "bv")}

    in_maps = []
    for c in range(8):
        b, hh = divmod(c, 2)
        hsl = slice(hh * NH, (hh + 1) * NH)
        csl = slice(hh * HOUT, (hh + 1) * HOUT)
        m = {
            "x": np.ascontiguousarray(hs[b]),
            "mask": np.ascontiguousarray(am[b, 0, 0]),
            "rel1": np.ascontiguousarray(rel1[b, hsl]),
            "rel2": np.ascontiguousarray(rel2[b, hsl]),
        }
        for k in ("wq", "wk", "wv"):
            m[k] = np.ascontiguousarray(ws[k][csl])
        for k in ("bq", "bk", "bv"):
            m[k] = np.ascontiguousarray(bs[k][csl])
        in_maps.append(m)
    return in_maps


def gather_output(results):
    out = np.empty((4, S, HIN), np.float32)
    for c in range(8):
        b, hh = divmod(c, 2)
        out[b, :, hh * HOUT:(hh + 1) * HOUT] = results[c]["out"]
    return out


_NC_CACHE = []


def kernel(**inputs):
    if not _NC_CACHE:
        _NC_CACHE.append(build_program())
    nc = _NC_CACHE[0]
    in_maps = make_in_maps(inputs)
    res = run_bass_kernel_spmd(nc, in_maps, list(range(8)))
    return gather_output(res.results)
